# revision 1
# baseline (speedup 1.0000x reference)
# CenterNet decode kernel for Trainium2 (Bass/Tile), 8-core data-parallel.
#
# Reference computation (per image):
#   heat = sigmoid(hm); heat *= (3x3 maxpool(heat) == heat)    # pool NMS
#   conf = max_c heat; cls = argmax_c heat
#   boxes from wh/reg + meshgrid; dets = [x1,y1,x2,y2,conf,cls] * (conf > 0.3)
#
# Device algorithm works in logit space (sigmoid is strictly monotone; one
# sigmoid at the end on the per-pixel winner):
#   - host pads hm to [C, H+2, W+2] with -1e30 so strip loads (incl. halo
#     rows and column pads) are single contiguous DMAs.
#   - phase 1 packs the flat (strip, channel) axis onto all 128 partitions:
#     per image, units u = s*80 + c.  Pooling and suppression are
#     unit-elementwise: separable 3x3 max, then w = x if x == pooled else
#     -FLT_MAX (custom DVE select op).
#   - phase 2: PE-transposes 128x128 pixel chunks into PSUM, ACT copies to
#     an SBUF staging tile, then per-strip-run channel max (conf, DVE
#     reduce with partial combines for strips whose channels span two
#     iterations).  argmax is computed without a second reduce: an
#     is_equal mask vs conf (both slices of a split strip fill one
#     combined [pix, chunk, 80] tile), PE-transposed back and contracted
#     on the PE against exact 2^(39-c) weights; since a sum of distinct
#     powers of two keeps the exponent of its largest term, the exponent
#     of the matmul result is 39 - argmax with first-index tie semantics,
#     recovered via ACT Ln and a guarded magic-number round on DVE.
#   - outputs in [128, G] layout with pixel = col*128 + p; the host
#     transposes wh/reg/meshgrid inputs to match and un-transposes dets.
import os
import sys
import threading

for _p in ("/opt/trn_rl_repo", "/root/.axon_site/_ro/trn_rl_repo"):
    if os.path.isdir(_p) and _p not in sys.path:
        sys.path.insert(0, _p)

import numpy as np

from concourse import bacc, bass, masks, mybir, tile

F32 = mybir.dt.float32
AX = mybir.AxisListType
OP = mybir.AluOpType
ACTF = mybir.ActivationFunctionType

NEG = -1e30
_FLT_MAX = float(np.finfo(np.float32).max)
_CUSTOM = {}


def _custom_ops():
    """CN_WSEL: w = x if x == pooled else -FLT_MAX
    CN_IDXC:  cand = (elem pos within page) if w == conf else s0"""
    if _CUSTOM:
        return _CUSTOM
    import re
    from concourse.dve_spec import (Spec, Src0, Src1, MaxNeg, select, eq,
                                    Idx, SubIdx, C0, C1)
    from concourse import dve_ops as D
    from concourse.dve_ops import DveOp, OPS

    def reg(name, spec, subdim):
        for op in OPS:
            if op.name == name:
                return op
        op = DveOp(name, spec, subdim=subdim, uops_sha={})
        OPS.append(op)
        D.CUSTOM_DVE_SPECS[name] = spec
        D._SUB_OPCODE_FOR_NAME[name] = D._CUSTOM_DVE_ROW_BASE + len(OPS) - 1
        for ver in ("v3", "v4"):
            try:
                op.compile(ver)
            except ValueError as e:
                m = re.search(r"%s: ([0-9a-f]+)" % ver, str(e))
                if m:
                    op.uops_sha[ver] = m.group(1)
                    op.compile(ver)
        return op

    _CUSTOM["wsel"] = reg(
        "CN_WSEL",
        Spec(body=select(eq(Src0, Src1), Src0, MaxNeg),
             reference=lambda in0, in1, c0=0, c1=0, c2=0: np.where(
                 in0 == in1.reshape(in0.shape), in0,
                 -_FLT_MAX).astype(in0.dtype)),
        subdim=False)
    _CUSTOM["idxc"] = reg(
        "CN_IDXC",
        Spec(body=select(eq(Src0, Src1), Idx - SubIdx * C1, C0),
             reference=lambda in0, in1, c0=0, c1=0, c2=0: np.where(
                 in0 == in1.reshape(in0.shape),
                 (np.arange(in0.shape[-2] * in0.shape[-1], dtype=np.float32)
                  .reshape(in0.shape[-2], in0.shape[-1])
                  - np.arange(in0.shape[-2], dtype=np.float32)[:, None]
                  * np.float32(np.asarray(c1).flat[0]))[None],
                 np.asarray(c0, dtype=np.float32)).astype(np.float32)),
        subdim=True)
    return _CUSTOM


def _plan(C=80, n_strips=16):
    """Static run/slice bookkeeping for the flat (s, c) unit axis."""
    total = C * n_strips
    assert total % 128 == 0
    n_iters = total // 128
    iter_runs = []       # per iter: list of (s, c0, c1, col0, col1)
    for j in range(n_iters):
        u0, u1 = 128 * j, 128 * j + 128
        runs = []
        for s in range(u0 // C, (u1 - 1) // C + 1):
            a, b = max(u0, s * C), min(u1, (s + 1) * C)
            runs.append((s, a - s * C, b - s * C, a - u0, b - u0))
        iter_runs.append(runs)
    # strip s completes in iter j_end
    strip_done = [((s + 1) * C - 1) // 128 for s in range(n_strips)]
    # slices per strip: (j, col0, col1, c0)
    strip_slices = [[] for _ in range(n_strips)]
    for j, runs in enumerate(iter_runs):
        for (s, c0, c1, col0, col1) in runs:
            strip_slices[s].append((j, col0, col1, c0))
    return n_iters, iter_runs, strip_done, strip_slices


def build_nc(Bc=4, C=80, H=256, W=256, S=16, n_devices=8, reps=1,
             use_gpsimd=False, h1_dma=False, pe_idx=True, v2_dma=0):
    n_strips = H // S
    CPS = (S * W) // 128          # pixel chunks per strip (32)
    G = n_strips * CPS            # output cols per image (512)
    Hp, Wp = H + 2, W + 2
    FL = (S + 2) * Wp             # per-unit input floats
    n_iters, iter_runs, strip_done, strip_slices = _plan(C, n_strips)
    TPG = 4                       # chunks per PSUM bank
    assert CPS % TPG == 0 and TPG * 128 * 4 <= 2048

    cust = _custom_ops()
    nc = bacc.Bacc("TRN2", target_bir_lowering=False, debug=False,
                   num_devices=n_devices)
    hmp = nc.dram_tensor("hmp", [Bc, C, Hp * Wp], F32, kind="ExternalInput")
    whr = nc.dram_tensor("whr", [Bc, 4, 128, G], F32, kind="ExternalInput")
    xyv = nc.dram_tensor("xyv", [3, 128, G], F32, kind="ExternalInput")
    dets = nc.dram_tensor("dets", [Bc, 128, G * 6], F32,
                          kind="ExternalOutput")

    eng1 = nc.gpsimd if use_gpsimd else nc.vector

    with tile.TileContext(nc) as tc:
        with (
            tc.tile_pool(name="singles", bufs=1) as singles,
            tc.tile_pool(name="xp", bufs=2) as xp_pool,
            tc.tile_pool(name="tmp", bufs=3) as tmp_pool,
            tc.tile_pool(name="vm", bufs=2) as vm_pool,
            tc.tile_pool(name="wv", bufs=1) as w_pool,
            tc.tile_pool(name="stg", bufs=2) as stg_pool,
            tc.tile_pool(name="im", bufs=1) as im_pool,
            tc.tile_pool(name="part", bufs=3) as part_pool,
            tc.tile_pool(name="res", bufs=2) as res_pool,
            tc.tile_pool(name="asm", bufs=1) as asm_pool,
            tc.tile_pool(name="oim", bufs=2) as oim_pool,
            tc.tile_pool(name="eqb", bufs=2) as eqb_pool,
            tc.tile_pool(name="psum_t", bufs=7 if pe_idx else 8,
                         space="PSUM") as psum_pool,
            tc.tile_pool(name="psum_s", bufs=1, space="PSUM") as psum_s_pool,
        ):
            ident = singles.tile([128, 128], F32)
            masks.make_identity(nc, ident[:])
            xvn = singles.tile([128, G], F32)
            yvn = singles.tile([128, G], F32)
            nc.sync.dma_start(xvn[:], xyv[0])
            nc.sync.dma_start(yvn[:], xyv[1])
            pw = singles.tile([128, 1], F32)
            nc.sync.dma_start(pw[:], xyv[2, :, 0:1])
            LN2 = float(np.log(2.0))
            MAGIC = 8388608.0

            def pe_idx_finalize(conf_s, idx_s, slices, staged):
                """idx via eq-mask -> PE transpose-back -> PE matmul with
                2^(C-1-c-40) weights; exponent(sum) = C-1-40-argmax
                (first-index, exact: sums of distinct powers of two keep
                the top exponent).  Both slices of a split strip write
                complementary channel ranges of one combined eq tile, so
                each chunk needs a single non-accumulating matmul."""
                ssum = psum_s_pool.tile([128, CPS], F32, tag="ssum")
                im = im_pool.tile([128, CPS * C], F32, tag="im")
                im3 = im[:].rearrange("p (k c) -> p k c", c=C)
                for (js, scol0, scol1, sc0) in slices:
                    ln = scol1 - scol0
                    src3 = staged[js % 2]
                    cb = conf_s.unsqueeze(-1).broadcast_to((128, CPS, ln))
                    nc.vector.tensor_tensor(
                        im3[:, :, sc0:sc0 + ln], src3[:, :, scol0:scol1],
                        cb, op=OP.is_equal)
                for g0 in range(0, CPS, TPG):
                    pt = psum_pool.tile([128, TPG * 128], F32, tag="pt")
                    for t in range(TPG):
                        k = g0 + t
                        nc.tensor.transpose(
                            pt[0:C, t * 128:(t + 1) * 128],
                            im3[:, k, :], ident[:])
                    eqb = eqb_pool.tile([128, TPG * 128], F32, tag="eqb")
                    nc.scalar.copy(eqb[0:C, :], pt[0:C, :])
                    for t in range(TPG):
                        k = g0 + t
                        nc.tensor.matmul(
                            ssum[:, k:k + 1],
                            eqb[0:C, t * 128:(t + 1) * 128],
                            pw[0:C, 0:1], start=True, stop=True,
                            skip_group_check=True)
                lnv = part_pool.tile([128, CPS], F32, tag="lnv")
                nc.scalar.activation(lnv[:, :], ssum[:, :], ACTF.Ln)
                nc.vector.tensor_scalar(
                    lnv[:, :], lnv[:, :], -1.0 / LN2,
                    float(C - 1 - 40) + 0.3, op0=OP.mult, op1=OP.add)
                nc.vector.tensor_scalar(
                    idx_s, lnv[:, :], MAGIC, MAGIC,
                    op0=OP.add, op1=OP.subtract)

            GB = min(G, 128)
            SPB = GB // CPS          # strips per assembly block
            block_ready = [max(strip_done[bk * SPB + s] for s in range(SPB))
                           for bk in range(G // GB)]

            for _rep in range(reps):
              for b in range(Bc):
                conf_g = res_pool.tile([128, G], F32, tag="conf_g")
                idx_g = res_pool.tile([128, G], F32, tag="idx_g")
                staged = [None, None]
                # partial conf tiles for strips split across iters
                pend = {}

                # ---- assembly for image b: emitted early, as soon as
                # the 4 strips covering each 128-col block finalize ----
                def assemble_block(g0):
                    gs = slice(g0, g0 + GB)
                    wh0 = asm_pool.tile([128, GB], F32, tag="wh0")
                    wh1 = asm_pool.tile([128, GB], F32, tag="wh1")
                    rg0 = asm_pool.tile([128, GB], F32, tag="rg0")
                    rg1 = asm_pool.tile([128, GB], F32, tag="rg1")
                    nc.sync.dma_start(wh0[:], whr[b, 0, :, gs])
                    nc.sync.dma_start(wh1[:], whr[b, 1, :, gs])
                    nc.sync.dma_start(rg0[:], whr[b, 2, :, gs])
                    nc.sync.dma_start(rg1[:], whr[b, 3, :, gs])

                    confs = asm_pool.tile([128, GB], F32, tag="confs")
                    nc.scalar.activation(confs[:], conf_g[:, gs],
                                         ACTF.Sigmoid)
                    mask = asm_pool.tile([128, GB], F32, tag="mask")
                    nc.vector.tensor_scalar(mask[:], confs[:], 0.3, None,
                                            op0=OP.is_gt)

                    out_b = oim_pool.tile([128, GB * 6], F32, tag="out_b")
                    o3 = out_b[:].rearrange("p (g k) -> p g k", k=6)

                    tcx, tcy, hwx, hwy = rg0, rg1, wh0, wh1
                    nc.vector.scalar_tensor_tensor(
                        tcx[:], rg0[:], 1.0 / W, xvn[:, gs], op0=OP.mult,
                        op1=OP.add)
                    nc.vector.scalar_tensor_tensor(
                        tcy[:], rg1[:], 1.0 / H, yvn[:, gs], op0=OP.mult,
                        op1=OP.add)
                    nc.vector.tensor_tensor(tcx[:], tcx[:], mask[:],
                                            op=OP.mult)
                    nc.vector.tensor_tensor(tcy[:], tcy[:], mask[:],
                                            op=OP.mult)
                    nc.vector.scalar_tensor_tensor(
                        hwx[:], wh0[:], 0.5 / W, mask[:], op0=OP.mult,
                        op1=OP.mult)
                    nc.vector.scalar_tensor_tensor(
                        hwy[:], wh1[:], 0.5 / H, mask[:], op0=OP.mult,
                        op1=OP.mult)

                    nc.vector.tensor_tensor(o3[:, :, 0], tcx[:], hwx[:],
                                            op=OP.subtract)
                    nc.vector.tensor_tensor(o3[:, :, 1], tcy[:], hwy[:],
                                            op=OP.subtract)
                    nc.vector.tensor_tensor(o3[:, :, 2], tcx[:], hwx[:],
                                            op=OP.add)
                    nc.vector.tensor_tensor(o3[:, :, 3], tcy[:], hwy[:],
                                            op=OP.add)
                    nc.vector.tensor_tensor(o3[:, :, 4], confs[:], mask[:],
                                            op=OP.mult)
                    nc.vector.tensor_tensor(o3[:, :, 5], idx_g[:, gs],
                                            mask[:], op=OP.mult)

                    nc.sync.dma_start(
                        dets[b, :, g0 * 6:(g0 + GB) * 6], out_b[:])


                for j in range(n_iters):
                    runs = iter_runs[j]
                    # ---- load: one DMA per run (contiguous per unit) ----
                    xp = xp_pool.tile([128, FL], F32, tag="xp")
                    for (s, c0, c1, col0, col1) in runs:
                        src = hmp[b, c0:c1,
                                  s * S * Wp:(s * S + S + 2) * Wp]
                        nc.sync.dma_start(xp[col0:col1, :], src)
                    xp3 = xp[:].rearrange("u (r w) -> u r w", w=Wp)

                    # ---- separable 3x3 max pool ----
                    m1 = tmp_pool.tile([128, (S + 2) * (W + 1)], F32,
                                       tag="pt")
                    m13 = m1[:].rearrange("u (r w) -> u r w", w=W + 1)
                    if h1_dma:
                        # H1 via DMA: plain load of cols 0..W, then CCE
                        # accum-max load of cols 1..W+1.
                        for (s, c0, c1, col0, col1) in runs:
                            src3 = hmp[b, c0:c1,
                                       s * S * Wp:(s * S + S + 2) * Wp] \
                                .rearrange("c (r w) -> c r w", w=Wp)
                            nc.sync.dma_start(
                                m13[col0:col1, :, :], src3[:, :, 0:W + 1])
                            nc.gpsimd.dma_start(
                                m13[col0:col1, :, :], src3[:, :, 1:W + 2],
                                accum_op=OP.max)
                    else:
                        eng1.tensor_tensor(
                            m13[:, :, :], xp3[:, :, 0:W + 1],
                            xp3[:, :, 1:W + 2], op=OP.max)
                    hx = tmp_pool.tile([128, (S + 2) * W], F32, tag="pt")
                    hx3 = hx[:].rearrange("u (r w) -> u r w", w=W)
                    nc.vector.tensor_tensor(
                        hx3[:, :, :], m13[:, :, 0:W], m13[:, :, 1:W + 1],
                        op=OP.max)
                    mv = tmp_pool.tile([128, (S + 1) * W], F32, tag="pt")
                    mv3 = mv[:].rearrange("u (r w) -> u r w", w=W)
                    nc.vector.tensor_tensor(
                        mv3[:, :, :], hx3[:, 0:S + 1, :], hx3[:, 1:S + 2, :],
                        op=OP.max)
                    vm = vm_pool.tile([128, S * W], F32, tag="vm")
                    vm3 = vm[:].rearrange("u (r w) -> u r w", w=W)
                    if v2_dma and (j + b * n_iters) % v2_dma == 0:
                        nc.sync.dma_start(vm[:], mv[:, 0:S * W])
                        nc.gpsimd.dma_start(vm[:], mv[:, W:(S + 1) * W],
                                            accum_op=OP.max)
                    else:
                        eng1.tensor_tensor(
                            vm3[:, :, :], mv3[:, 0:S, :], mv3[:, 1:S + 1, :],
                            op=OP.max)

                    # ---- suppression ----
                    wv = w_pool.tile([128, S * W], F32, tag="wv")
                    wv3 = wv[:].rearrange("u (r w) -> u r w", w=W)
                    nc.vector._custom_dve(cust["wsel"], out=wv3[:, :, :],
                                          in0=xp3[:, 1:S + 1, 1:W + 1],
                                          in1=vm3[:, :, :])

                    # ---- transpose chunks -> PSUM -> staged SBUF ----
                    stg = stg_pool.tile([128, CPS * 128], F32, tag="stg")
                    stg3 = stg[:].rearrange("p (k u) -> p k u", u=128)
                    for g0 in range(0, CPS, TPG):
                        pt = psum_pool.tile([128, TPG * 128], F32, tag="pt")
                        pt3 = pt[:].rearrange("p (t u) -> p t u", u=128)
                        for t in range(TPG):
                            k = g0 + t
                            nc.tensor.transpose(
                                pt3[:, t, :],
                                wv[:, k * 128:(k + 1) * 128], ident[:])
                        nc.scalar.copy(
                            stg[:, g0 * 128:(g0 + TPG) * 128], pt[:])
                    staged[j % 2] = stg3

                    # ---- per-run partial conf (direct when 1-slice) ----
                    red = nc.vector
                    for (s, c0, c1, col0, col1) in runs:
                        single = len(strip_slices[s]) == 1
                        if single:
                            cp = None
                            dst = conf_g[:, s * CPS:(s + 1) * CPS]
                        else:
                            cp = part_pool.tile([128, CPS], F32,
                                                tag=f"cp{s % 3}")
                            dst = cp[:, :]
                        red.tensor_reduce(
                            dst, stg3[:, :, col0:col1], axis=AX.X,
                            op=OP.max)
                        if not single:
                            pend.setdefault(s, []).append(cp)

                    # ---- finalize strips completed this iter ----
                    for (s, c0, c1, col0, col1) in runs:
                        if strip_done[s] != j:
                            continue
                        conf_s = conf_g[:, s * CPS:(s + 1) * CPS]
                        idx_s = idx_g[:, s * CPS:(s + 1) * CPS]
                        slices = strip_slices[s]
                        if len(slices) > 1:
                            parts = pend.pop(s)
                            nc.vector.tensor_tensor(
                                conf_s, parts[0][:, :], parts[1][:, :],
                                op=OP.max)
                        if pe_idx:
                            pe_idx_finalize(conf_s, idx_s, slices, staged)
                            continue
                        imins = []
                        for (js, scol0, scol1, sc0) in slices:
                            ln = scol1 - scol0
                            src3 = staged[js % 2]
                            im = im_pool.tile([128, CPS * ln], F32,
                                              tag="im")
                            im3 = im[:].rearrange("p (k c) -> p k c", c=ln)
                            cb = conf_s.unsqueeze(-1).broadcast_to(
                                (128, CPS, ln))
                            nc.vector._custom_dve(
                                cust["idxc"], out=im3[:, :, :],
                                in0=src3[:, :, scol0:scol1], in1=cb,
                                s0=1e4 - sc0, s1=float(ln))
                            if len(slices) == 1:
                                red.tensor_reduce(
                                    idx_s, im3[:, :, :], axis=AX.X,
                                    op=OP.min)
                            else:
                                imt = part_pool.tile(
                                    [128, CPS], F32,
                                    tag=f"imt{len(imins)}")
                                red.tensor_reduce(
                                    imt[:, :], im3[:, :, :], axis=AX.X,
                                    op=OP.min)
                                if sc0:
                                    nc.vector.tensor_scalar(
                                        imt[:, :], imt[:, :], float(sc0),
                                        None, op0=OP.add)
                                imins.append(imt)
                        if len(slices) > 1:
                            nc.vector.tensor_tensor(
                                idx_s, imins[0][:, :], imins[1][:, :],
                                op=OP.min)

                    for bk in range(G // GB):
                        if block_ready[bk] == j:
                            assemble_block(bk * GB)


                for g0 in range(0, G, GB):
                    if block_ready[g0 // GB] >= n_iters:
                        assemble_block(g0)

    nc.compile()
    return nc


def prep_core_inputs(hm_c, wh_c, reg_c):
    """Host-side prep for one core: pad hm, transpose wh/reg/xyv/meshgrid
    into the [128, G] pixel = col*128 + p layout."""
    Bc, C, H, W = hm_c.shape
    G = (H * W) // 128
    hmp = np.full((Bc, C, H + 2, W + 2), NEG, np.float32)
    hmp[:, :, 1:H + 1, 1:W + 1] = hm_c
    # pixel = col*128 + p  ->  [128, G] view = reshape(G,128).T
    def t(x):   # [H*W] -> [128, G]
        return np.ascontiguousarray(x.reshape(G, 128).T)
    whr = np.empty((Bc, 4, 128, G), np.float32)
    for b in range(Bc):
        whr[b, 0] = t(wh_c[b, 0].reshape(-1))
        whr[b, 1] = t(wh_c[b, 1].reshape(-1))
        whr[b, 2] = t(reg_c[b, 0].reshape(-1))
        whr[b, 3] = t(reg_c[b, 1].reshape(-1))
    yv, xv = np.meshgrid(np.arange(H, dtype=np.float32),
                         np.arange(W, dtype=np.float32), indexing="ij")
    pwp = np.zeros((128, G), np.float32)
    ii, oo = np.meshgrid(np.arange(128), np.arange(128), indexing="ij")
    ee = C - 1 - ii - oo - 40
    pwp[:, 0:128] = np.where(ee >= -126, np.ldexp(1.0, ee), 0.0
                             ).astype(np.float32)
    xyv = np.stack([t(xv.reshape(-1) / W), t(yv.reshape(-1) / H), pwp])
    return {"hmp": hmp.reshape(Bc, C, (H + 2) * (W + 2)),
            "whr": whr, "xyv": xyv.astype(np.float32)}


def post_dets(dev_out, H=256, W=256):
    """[Bc, 128, G*6] device layout -> [Bc, H*W, 6]."""
    Bc = dev_out.shape[0]
    G = (H * W) // 128
    d = dev_out.reshape(Bc, 128, G, 6).transpose(0, 2, 1, 3)
    return np.ascontiguousarray(d.reshape(Bc, H * W, 6))


_CACHE = {}
_CACHE_LOCK = threading.Lock()


def _get_nc(key, **kw):
    with _CACHE_LOCK:
        if key not in _CACHE:
            _CACHE[key] = build_nc(**kw)
        return _CACHE[key]


def kernel(hm: np.ndarray, wh: np.ndarray, reg: np.ndarray) -> np.ndarray:
    from concourse.bass_utils import run_bass_kernel_spmd

    B, C, H, W = hm.shape
    n_cores = 8
    assert B % n_cores == 0
    Bc = B // n_cores
    nc = _get_nc(("v3", Bc, C, H, W), Bc=Bc, C=C, H=H, W=W)
    in_maps = []
    for i in range(n_cores):
        sl = slice(i * Bc, (i + 1) * Bc)
        in_maps.append(prep_core_inputs(hm[sl], wh[sl], reg[sl]))
    res = run_bass_kernel_spmd(nc, in_maps, core_ids=list(range(n_cores)))
    return np.concatenate(
        [post_dets(res.results[i]["dets"], H, W) for i in range(n_cores)],
        axis=0)

